# revision 12
# baseline (speedup 1.0000x reference)
"""KMEAttention Trainium2 kernel — 8-core SPMD, no collectives.

Sharding: 8 cores = 4 batches x 2 query-halves. Each core computes one
(b, q-half): full k/v work for its b is duplicated across the pair, which
avoids any cross-core reduction (head dim stays whole per core).

Algebraic restructuring (host precomputes tiny weight fusions only):
  Gq[h,f,d]  = sum_e Wq[h*D+e, d] * freqs[h,e,f]   (proj+freq fused)
  Cmat[h,d,c]= sum_e Wv[h*D+e, d] * Wo[c, h*D+e]   (v-proj+o-proj fused)
so on device:  p = X @ Gq^T,  phi = [cos(p)|sin(p)],  z = sum_m w_m*phi_m,
scoresT = zk @ zq^T (1/256 scale folded into w_q), E = exp, AV^T = Xv^T @ E,
normalize by column sums, out_atoms = sum_h AV_h^T-contraction with Cmat.
"""

import sys

sys.path.insert(0, "/opt/trn_rl_repo")

import numpy as np

B, S, M, D, H, NF = 4, 1024, 8, 64, 8, 32
SQ = S // 2          # queries per core
F2 = 2 * NF          # 64: [cos|sin]
HF = H * F2          # 512
NQT = SQ * M // 128  # 32 q phi tiles
NKT = S * M // 128   # 64 k phi tiles

_CACHE = {}


def _build_program():
    import concourse.bacc as bacc
    import concourse.bass as bass
    import concourse.mybir as mybir
    import concourse.tile as tile

    fp32 = mybir.dt.float32
    f32r = mybir.dt.float32r
    AF = mybir.ActivationFunctionType

    nc = bacc.Bacc(trn_type="TRN2")

    # ---- dram I/O ----
    xqT = nc.dram_tensor("xqT", [2 * D, SQ * M // 2], f32r, kind="ExternalInput")
    xkT = nc.dram_tensor("xkT", [2 * D, S * M // 2], f32r, kind="ExternalInput")
    xv = nc.dram_tensor("xv", [S, M * D], f32r, kind="ExternalInput")
    qlw = nc.dram_tensor("qlw", [SQ, M], fp32, kind="ExternalInput")
    klw = nc.dram_tensor("klw", [S, M], fp32, kind="ExternalInput")
    gqT = nc.dram_tensor("gqT", [2 * D, H * NF], f32r, kind="ExternalInput")
    gkT = nc.dram_tensor("gkT", [2 * D, H * NF], f32r, kind="ExternalInput")
    c2 = nc.dram_tensor("c2", [H, 128, 128], f32r, kind="ExternalInput")
    wwT8 = nc.dram_tensor("wwT8", [D, M], f32r, kind="ExternalInput")
    eblk = nc.dram_tensor("eblk", [8, 128, 128], f32r, kind="ExternalInput")
    ident = nc.dram_tensor("ident", [128, 128], f32r, kind="ExternalInput")
    ones_d = nc.dram_tensor("ones_d", [128, 1], f32r, kind="ExternalInput")
    wscr = nc.dram_tensor("wscr", [(SQ + S) * M], fp32)  # scratch round-trip
    dscr = nc.dram_tensor("dscr", [H, SQ], fp32)  # denom recip bounce
    out_a = nc.dram_tensor("out_a", [SQ, M * D], fp32, kind="ExternalOutput")
    out_n = nc.dram_tensor("out_n", [SQ, M], fp32, kind="ExternalOutput")

    with tile.TileContext(nc) as tc, nc.allow_low_precision(
            reason="f32r transposes; all matmul accumulation is fp32 PSUM"):
        with (
            tc.tile_pool(name="const", bufs=1) as constp,
            tc.tile_pool(name="big", bufs=1) as bigp,
            tc.tile_pool(name="work", bufs=3) as workp,
            tc.tile_pool(name="escr", bufs=1) as escr,
            tc.tile_pool(name="oap", bufs=1) as oap,
        ):
            # ---- persistent SBUF ----
            qkpool = tc.tile_pool(name="qkp", bufs=1)
            qkp = qkpool.__enter__()
            sb_xqT = qkp.tile([2 * D, SQ * M // 2], f32r, tag="xqT")
            sb_xkT = qkp.tile([2 * D, S * M // 2], f32r, tag="xkT")
            sb_xv = [bigp.tile([128, M * D], f32r, tag=f"xv{j}", name=f"xv{j}") for j in range(8)]
            sb_gqT = constp.tile([2 * D, H * NF], f32r)
            sb_gkT = constp.tile([2 * D, H * NF], f32r)
            sb_c2 = [constp.tile([128, 128], f32r, tag=f"c2_{h}", name=f"c2_{h}") for h in range(H)]
            sb_wwT8 = constp.tile([D, M], f32r)
            sb_eblk = [constp.tile([128, 128], f32r, tag=f"eb{j}", name=f"eb{j}") for j in range(8)]
            sb_id = constp.tile([128, 128], f32r)
            sb_ones = constp.tile([128, 1], f32r)
            nc.sync.dma_start(out=sb_ones, in_=ones_d[:, :])
            sb_pihalf = constp.tile([128, 1], fp32)
            nc.vector.memset(sb_pihalf, float(np.pi / 2))

            nc.sync.dma_start(out=sb_xqT, in_=xqT[:, :])
            nc.sync.dma_start(out=sb_xkT, in_=xkT[:, :])
            for j in range(8):
                nc.sync.dma_start(out=sb_xv[j], in_=xv[j * 128:(j + 1) * 128, :])
            nc.sync.dma_start(out=sb_gqT, in_=gqT[:, :])
            nc.sync.dma_start(out=sb_gkT, in_=gkT[:, :])
            for h in range(H):
                nc.sync.dma_start(out=sb_c2[h], in_=c2[h, :, :])
            nc.sync.dma_start(out=sb_wwT8, in_=wwT8[:, :])
            for j in range(8):
                nc.sync.dma_start(out=sb_eblk[j], in_=eblk[j, :, :])
            nc.sync.dma_start(out=sb_id, in_=ident[:, :])

            # ---- phase 1: softmax weights (exp table) ----
            # natural [s,m] layout; fold 1/256 into q-side recips
            for (lw, nst, qside) in ((qlw, SQ // 128, True), (klw, S // 128, False)):
                for t in range(nst):
                    lt = workp.tile([128, M], fp32, tag="lw")
                    nc.sync.dma_start(out=lt, in_=lw[t * 128:(t + 1) * 128, :])
                    ex = workp.tile([128, M], fp32, tag="lwex")
                    nc.scalar.activation(out=ex, in_=lt, func=AF.Exp)
                    sm = workp.tile([128, 1], fp32, tag="lwsum")
                    nc.vector.reduce_sum(out=sm, in_=ex, axis=mybir.AxisListType.X)
                    rc = workp.tile([128, 1], fp32, tag="lwrec")
                    nc.vector.reciprocal(out=rc, in_=sm)
                    if qside:
                        nc.scalar.mul(out=rc, in_=rc, mul=1.0 / 256.0)
                    wt = workp.tile([128, M], fp32, tag="lww")
                    nc.vector.tensor_scalar_mul(out=wt, in0=ex, scalar1=rc)
                    off = (0 if qside else SQ * M) + t * 128 * M
                    nc.sync.dma_start(
                        out=wscr[off:off + 128 * M].rearrange("(p m) -> p m", p=128),
                        in_=wt)
            # read back as [(s,m)] columns
            sb_wcq = bigp.tile([128, NQT], fp32, tag="wcq")
            sb_wck = bigp.tile([128, NKT], fp32, tag="wck")
            nc.sync.dma_start(
                out=sb_wcq, in_=wscr[:SQ * M].rearrange("(t p) -> p t", p=128))
            nc.sync.dma_start(
                out=sb_wck, in_=wscr[SQ * M:].rearrange("(t p) -> p t", p=128))

            # ---- phase 2: p -> phi -> wphi -> z -> zT  (trig table) ----
            sb_zqT = [bigp.tile([128, SQ], f32r, tag=f"zqT{t}", name=f"zqT{t}") for t in range(4)]
            sb_zkT = [bigp.tile([128, S], f32r, tag=f"zkT{t}", name=f"zkT{t}") for t in range(4)]

            ps2 = tc.tile_pool(name="ps2", bufs=2, space="PSUM")
            psp = pszp = ps2.__enter__()
            for (xT, gT, wcol, nt, zT_tiles, nzt) in (
                (sb_xqT, sb_gqT, sb_wcq, NQT, sb_zqT, 4),
                (sb_xkT, sb_gkT, sb_wck, NKT, sb_zkT, 8),
            ):
                for zt in range(nzt):  # one z tile = 128 s rows
                    zps = pszp.tile([128, HF], fp32, tag="zacc")
                    for j in range(8):
                        t = zt * 8 + j
                        half2 = t >= nt // 2
                        r0 = D if half2 else 0
                        tc2 = (t - nt // 2) if half2 else t
                        pps = psp.tile([128, H * NF], fp32, tag="p")
                        nc.tensor.matmul(
                            out=pps,
                            lhsT=xT[r0:r0 + D, tc2 * 128:(tc2 + 1) * 128],
                            rhs=gT[r0:r0 + D, :], start=True, stop=True)
                        phi = workp.tile([128, HF], fp32, tag="phi")
                        phv = phi.rearrange("p (h f) -> p h f", h=H)
                        nc.scalar.activation(
                            out=phv[:, :, 0:NF], in_=pps, func=AF.Sin,
                            bias=sb_pihalf, scale=1.0)
                        nc.scalar.activation(
                            out=phv[:, :, NF:F2], in_=pps, func=AF.Sin)
                        wphi = workp.tile([128, HF], f32r, tag="wphi")
                        nc.vector.tensor_scalar_mul(
                            out=wphi, in0=phi, scalar1=wcol[:, t:t + 1])
                        nc.tensor.matmul(
                            out=zps, lhsT=sb_eblk[j],
                            rhs=wphi,
                            start=(j == 0), stop=(j == 7))
                    zsb = workp.tile([128, HF], f32r, tag="znat")
                    nc.vector.tensor_copy(out=zsb, in_=zps)
                    for c in range(4):  # transpose 128x128 chunks
                        tps = psp.tile([128, 128], f32r, tag="ztp")
                        nc.tensor.transpose(
                            out=tps, in_=zsb[:, c * 128:(c + 1) * 128], identity=sb_id)
                        nc.vector.tensor_copy(
                            out=zT_tiles[c][:, zt * 128:(zt + 1) * 128], in_=tps)

            ps2.__exit__(None, None, None)
            qkpool.__exit__(None, None, None)
            ps3 = tc.tile_pool(name="ps3", bufs=1, space="PSUM")
            avp = ps3.__enter__()
            ps3b = tc.tile_pool(name="ps3b", bufs=2, space="PSUM")
            psp = pszp = ps3b.__enter__()
            # ---- phase 3: attention per head (exp table) ----
            sb_avn = [[escr.tile([128, SQ], f32r, tag=f"avn{h}_{t}", name=f"avn{h}_{t}")
                       for t in range(4)] for h in range(H)]
            for h in range(H):
                pt, off = h // 2, (h % 2) * 64
                av = [avp.tile([128, SQ], fp32, tag=f"av{t}", name=f"av{t}") for t in range(4)]
                den = psp.tile([1, SQ], fp32, tag="den")
                for j in range(8):  # k tiles
                    sc = pszp.tile([128, SQ], fp32, tag="sc")
                    nc.tensor.matmul(
                        out=sc,
                        lhsT=sb_zkT[pt][off:off + 64, j * 128:(j + 1) * 128],
                        rhs=sb_zqT[pt][off:off + 64, :],
                        start=True, stop=True)
                    et = workp.tile([128, SQ], f32r, tag="et")
                    nc.scalar.activation(out=et, in_=sc, func=AF.Exp)
                    for t in range(4):
                        nc.tensor.matmul(
                            out=av[t],
                            lhsT=sb_xv[j][:, t * 128:(t + 1) * 128],
                            rhs=et, start=(j == 0), stop=(j == 7))
                    nc.tensor.matmul(
                        out=den, lhsT=sb_ones, rhs=et,
                        start=(j == 0), stop=(j == 7))
                drc = workp.tile([1, SQ], fp32, tag="drc")
                nc.vector.reciprocal(out=drc, in_=den)
                dbc = workp.tile([128, SQ], fp32, tag="dbc")
                nc.sync.dma_start(out=dscr[h:h + 1, :], in_=drc)
                nc.sync.dma_start(
                    out=dbc, in_=dscr[h:h + 1, :].to_broadcast((128, SQ)))
                for t in range(4):
                    nc.vector.tensor_mul(out=sb_avn[h][t], in0=av[t], in1=dbc)

            ps3b.__exit__(None, None, None)
            ps3.__exit__(None, None, None)
            ps4 = tc.tile_pool(name="ps4", bufs=2, space="PSUM")
            psp = avp = ps4.__enter__()
            # ---- phase 4: C-stage -> out_atoms, feat, nlw ----
            sb_oa = [oap.tile([128, M * D], fp32, tag=f"oa{qc}", name=f"oa{qc}") for qc in range(4)]
            sb_feat = workp.tile([128, 4 * D], f32r, tag="feat")  # cols: qc*64+c
            for qc in range(4):
                oaps = avp.tile([128, M * D], fp32, tag="oaps")
                for t in range(4):
                    for h in range(H):
                        nc.tensor.matmul(
                            out=oaps[:, t * 128:(t + 1) * 128],
                            lhsT=sb_avn[h][t][:, qc * 128:(qc + 1) * 128],
                            rhs=sb_c2[h],
                            start=(h == 0), stop=(h == H - 1))
                nc.vector.tensor_copy(out=sb_oa[qc], in_=oaps)
                nc.sync.dma_start(
                    out=out_a[qc * 128:(qc + 1) * 128, :], in_=sb_oa[qc])
                # feat = (1/M) sum_m oa  (the 1/M is folded into wwT8)
                nc.vector.reduce_sum(
                    out=sb_feat[:, qc * D:(qc + 1) * D],
                    in_=sb_oa[qc].rearrange("p (m c) -> p c m", m=M),
                    axis=mybir.AxisListType.X)
            # featT via PE transpose, then nlwT = wwT8^T-style matmul
            sb_ftT = workp.tile([D, SQ], f32r, tag="ftT")
            for qc in range(4):
                tps = psp.tile([128, 128], f32r, tag="ftp")
                nc.tensor.transpose(
                    out=tps[0:D, :], in_=sb_feat[:, qc * D:(qc + 1) * D],
                    identity=sb_id)
                nc.vector.tensor_copy(
                    out=sb_ftT[:, qc * 128:(qc + 1) * 128], in_=tps[0:D, :])
            nlwT = psp.tile([M, SQ], fp32, tag="nlwT")
            nc.tensor.matmul(out=nlwT, lhsT=sb_wwT8,
                             rhs=sb_ftT, start=True, stop=True)
            sb_nlwT = workp.tile([M, SQ], f32r, tag="nlwTs")
            nc.vector.tensor_copy(out=sb_nlwT, in_=nlwT)
            for qc in range(4):
                tps = psp.tile([128, 128], f32r, tag="ntp")
                nc.tensor.transpose(
                    out=tps[0:128, 0:M],
                    in_=sb_nlwT[:, qc * 128:(qc + 1) * 128],
                    identity=sb_id[0:M, 0:M])
                lt = workp.tile([128, M], fp32, tag="lwq2")
                nc.sync.dma_start(out=lt, in_=qlw[qc * 128:(qc + 1) * 128, :])
                nl = workp.tile([128, M], fp32, tag="nlo")
                nc.vector.tensor_add(out=nl, in0=tps[0:128, 0:M], in1=lt)
                nc.sync.dma_start(out=out_n[qc * 128:(qc + 1) * 128, :], in_=nl)
            ps4.__exit__(None, None, None)

    nc.finalize()
    return nc


def _stackT(X):
    XT = X.T  # [D, N]
    n2 = XT.shape[1] // 2
    return np.ascontiguousarray(
        np.concatenate([XT[:, :n2], XT[:, n2:]], axis=0), np.float32)


def _host_prep(q_atoms, q_logw, k_atoms, k_logw, v_atoms, v_logw,
               Wq, Wk, Wv, Wo, Ww, freqs):
    f32 = np.float32
    Gq = np.einsum('hed,hef->hfd', Wq.reshape(H, D, D), freqs)
    Gk = np.einsum('hed,hef->hfd', Wk.reshape(H, D, D), freqs)
    gqT = np.ascontiguousarray(
        np.tile(Gq.reshape(H * NF, D).T, (2, 1)), f32)
    gkT = np.ascontiguousarray(
        np.tile(Gk.reshape(H * NF, D).T, (2, 1)), f32)
    Cmat = np.einsum('hed,che->hdc', Wv.reshape(H, D, D), Wo.reshape(D, H, D))
    c2 = np.zeros((H, 128, 128), f32)
    c2[:, :D, :D] = Cmat
    c2[:, D:, D:] = Cmat
    wwT8 = np.ascontiguousarray(Ww.T / M, f32)
    eblk = np.zeros((8, 128, 128), f32)
    for j in range(8):
        for r in range(128):
            eblk[j, r, 16 * j + r // 8] = 1.0
    ident = np.eye(128, dtype=f32)
    ins = []
    for c in range(8):
        b, half = c // 2, c % 2
        qs = half * SQ
        ins.append({
            "xqT": _stackT(q_atoms[b, qs:qs + SQ].reshape(SQ * M, D)),
            "xkT": _stackT(k_atoms[b].reshape(S * M, D)),
            "xv": np.ascontiguousarray(v_atoms[b].reshape(S, M * D), f32),
            "qlw": np.ascontiguousarray(q_logw[b, qs:qs + SQ], f32),
            "klw": np.ascontiguousarray(k_logw[b], f32),
            "gqT": gqT, "gkT": gkT, "c2": c2, "wwT8": wwT8,
            "eblk": eblk, "ident": ident, "ones_d": np.ones((128, 1), f32),
        })
    return ins


def kernel(q_atoms, q_logw, k_atoms, k_logw, v_atoms, v_logw,
           Wq, Wk, Wv, Wo, Ww, freqs, _trace=False):
    from concourse.bass_utils import run_bass_kernel_spmd

    if "nc" not in _CACHE:
        _CACHE["nc"] = _build_program()
    nc = _CACHE["nc"]
    ins = _host_prep(q_atoms, q_logw, k_atoms, k_logw, v_atoms, v_logw,
                     Wq, Wk, Wv, Wo, Ww, freqs)
    res = run_bass_kernel_spmd(nc, ins, list(range(8)), trace=_trace)
    out_atoms = np.zeros((B, S, M, D), np.float32)
    nlw = np.zeros((B, S, M), np.float32)
    for c in range(8):
        b, half = c // 2, c % 2
        qs = half * SQ
        out_atoms[b, qs:qs + SQ] = res.results[c]["out_a"].reshape(SQ, M, D)
        nlw[b, qs:qs + SQ] = res.results[c]["out_n"]
    if _trace:
        _CACHE["exec_time_ns"] = res.exec_time_ns
    return out_atoms, nlw
